# revision 30
# baseline (speedup 1.0000x reference)
"""Trainium2 Bass kernel for causal GQA attention (B=2, S=2048, D=2048,
H=32, KVH=8, hd=64) with RoPE and output projection, running SPMD on 8
NeuronCores.

Sharding: tensor-parallel over heads (4-way) x data-parallel over batch
(2-way).  Core c (b = c//4, k = c%4) handles batch b and heads
8k..8k+8 (kv heads 2k, 2k+1).  No collectives: each core computes a
PARTIAL wo product (contraction over its local 512 attention features,
all 2048 output dims) and the host sums the 4 partials per batch.

Layouts: everything lives in transposed [feature, seq] form so that the
head dim (the contraction dim of QK^T) sits on SBUF partitions and no
on-device transposes are required (except a cheap PE transpose for V).
All matmul operands are bf16 (stationary bf16 enables fast weight load
so LDWEIGHTS hides under the matmuls); accumulation is fp32 in PSUM.

Head pairing: q_fin[i] holds head 8k+i (kv head 2k) on partitions 0:64
and head 8k+4+i (kv head 2k+1) on partitions 64:128, so the two packed
score matmuls of a pair use the two DIFFERENT kv heads and the K
projection needs no duplication.

Score-matmul concurrency: the two half-head (contraction-64) score
matmuls of a kv tile sit on disjoint PE row groups (base partitions 0
and 64), so the hardware can run them CONCURRENTLY -- but only if they
are issued back-to-back.  To make that structural, each kv tile i gets
ONE psum tile [P, 2(hf), QT]; both hf matmuls write different banks of
it and become ready on the same exp-drain event, so they always emit
adjacently and overlap (~2x on the score phase).

Causal diagonal trimming: for q tile t, the last kv group (columns
t*512+256 .. (t+1)*512) is only needed by the second half of the q
rows, so its scores/exp/PV run at free dim 256.  The mask multiply
only ever touches the first 256 q columns of a group.

Schedule: the projection work for s-tile t+1 and the wo matmuls for
q-tile t-1 are WOVEN between the attention score groups of q-tile t, so
the PE has dense matmul work while the scalar engine (exp, the
second-busiest engine) chews through the softmax.  V transposes are
deferred a unit so they never head-of-line-block the PE queue.  PSUM:
score tag 2x2 banks + PV 2x1 + a shared 1-bank tag for projection
passes / wo chunks x2 = 8 banks.
"""

import numpy as np

DIM = 2048
S = 2048
B = 2
H = 32
KVH = 8
HD = 64
P = 128
QT = 512        # q tile (free dim of score matmuls)
QH = QT // 2    # half tile for the causal-diagonal group
ROPE_BASE = 10000.0
N_CORES = 8

_CACHE = {}


def _build(s_len=S):
    import concourse.bacc as bacc
    import concourse.tile as tile
    import concourse.mybir as mybir
    from concourse.masks import make_identity

    F32 = mybir.dt.float32
    BF16 = mybir.dt.bfloat16
    Exp = mybir.ActivationFunctionType.Exp

    nqt = s_len // QT      # q tiles
    nkv = s_len // P       # kv tiles of 128
    DK = DIM // P          # 16 contraction tiles for projections
    NXC = 4                # x chunks per s-tile
    OCH = DK // NXC        # 4 contraction 128-tiles per x chunk

    nc = bacc.Bacc("TRN2", target_bir_lowering=False, debug=False,
                   num_devices=N_CORES)

    # All DRAM tensors are HOST-PRE-TILED so that every DMA reads/writes
    # a long contiguous run per partition (2-4 KiB packets); the naive
    # [feature, seq] layouts generate 256 B packets and run the HWDGE
    # queues at <100 GB/s, which starves the prologue.
    nst = s_len // QT
    xT4 = nc.dram_tensor("xT4", [P, nst, NXC, OCH * QT], BF16,
                         kind="ExternalInput").ap()
    wqT4 = nc.dram_tensor("wqT4", [P, 4, DK, P], BF16,
                          kind="ExternalInput").ap()
    wkT3 = nc.dram_tensor("wkT3", [P, DK, P], BF16,
                          kind="ExternalInput").ap()
    wvT3 = nc.dram_tensor("wvT3", [P, DK, P], BF16,
                          kind="ExternalInput").ap()
    woT3 = nc.dram_tensor("woT3", [P, 4, DIM], BF16,
                          kind="ExternalInput").ap()
    cosT = nc.dram_tensor("cosT", [P, s_len], BF16, kind="ExternalInput").ap()
    sinT = nc.dram_tensor("sinT", [P, s_len], BF16, kind="ExternalInput").ap()
    maskT = nc.dram_tensor("maskT", [P, 2, 2, QH], BF16,
                           kind="ExternalInput").ap()
    rotmT = nc.dram_tensor("rotmT", [P, P], BF16, kind="ExternalInput").ap()
    out_td = nc.dram_tensor("out_td", [P, nst, DK, QT], BF16,
                            kind="ExternalOutput").ap()

    with tile.TileContext(nc) as tc:
        with (
            tc.tile_pool(name="pers", bufs=1) as pers,
            tc.tile_pool(name="ps", bufs=1, space="PSUM") as ps,
            tc.tile_pool(name="pc", bufs=1) as pc,
        ):
            # ---- persistent tiles ----
            q_fin = [pers.tile([P, s_len], BF16, name=f"q_fin{m}")
                     for m in range(4)]
            k_fin = pers.tile([P, s_len], BF16, name="k_fin")
            v1 = [pers.tile([P, nkv, P], BF16, name=f"v1_{g}")
                  for g in range(2)]
            a_fin = [pers.tile([P, s_len], BF16, name=f"a_fin{i}")
                     for i in range(4)]
            msk = pers.tile([P, 2, 2, QH], BF16, name="msk")
            vT_raw = pers.tile([P, s_len], BF16, name="vT_raw")
            wq_sb = [pers.tile([P, DK, P], BF16, name=f"wq_sb{m}")
                     for m in range(4)]
            wk_sb = pers.tile([P, DK, P], BF16, name="wk_sb")
            wv_sb = pers.tile([P, DK, P], BF16, name="wv_sb")
            wo_sb = pers.tile([P, 4, DIM], BF16, name="wo_sb")
            cos_sb = pers.tile([P, s_len], BF16, name="cos_sb")
            sin_sb = pers.tile([P, s_len], BF16, name="sin_sb")
            ident = pers.tile([P, P], BF16, name="ident")
            rope_mat = pers.tile([P, P], BF16, name="rope_mat")

            # PSUM tags (8 banks): sc2 = 2 tiles x 2 banks (score kv
            # tiles: [hf, q]), pv = 2 tiles x 1 bank (PV accum), aq = 2
            # tiles x 1 bank (projection passes, wo chunks, V
            # transposes).
            def sc2(name):
                return ps.tile([P, 2, QT], F32, tag="sc2", bufs=2, name=name)

            def pvb(name):
                return ps.tile([P, QT], F32, tag="pv", bufs=2, name=name)

            def aqb(name, shape=None, dtype=None):
                return ps.tile(shape or [P, QT], dtype or F32, tag="aq",
                               bufs=2, name=name)

            # ---------------- prologue DMAs ----------------
            # x chunks stream on the sync HWDGE queue; everything else on
            # the scalar HWDGE queue so the x stream is never stuck behind
            # 6 MB of weights.  Weight order matches the prologue pass
            # order (k, v, q0..q3): wk first so pass k starts ~1.5us in,
            # cos/sin right behind it so the k rope (which gates the first
            # score matmul) is never the long pole.
            xsl = {}

            def x_load(st):
                for cn in range(NXC):
                    t_ = pc.tile([P, OCH, QT], BF16, tag="xsl",
                                 bufs=2 * NXC, name=f"x{st}_{cn}")
                    nc.sync.dma_start(
                        t_[:],
                        xT4[:, st, cn].rearrange("p (o q) -> p o q", o=OCH))
                    xsl[st, cn] = t_

            x_load(0)
            nc.scalar.dma_start(wk_sb[:], wkT3[:])
            nc.scalar.dma_start(rope_mat[:], rotmT[:])
            nc.scalar.dma_start(cos_sb[:], cosT[:])
            nc.scalar.dma_start(sin_sb[:], sinT[:])
            nc.scalar.dma_start(wv_sb[:], wvT3[:])
            nc.scalar.dma_start(wq_sb[0][:], wqT4[:, 0])
            nc.scalar.dma_start(msk[:], maskT[:])
            for m in range(1, 4):
                nc.scalar.dma_start(wq_sb[m][:], wqT4[:, m])
            nc.scalar.dma_start(wo_sb[:], woT3[:])
            ident_f = pc.tile([P, P], F32, name="ident_f")
            make_identity(nc, ident_f[:])
            nc.vector.tensor_copy(ident[:], ident_f[:])
            ones3 = pc.tile([P, nkv, HD], F32, name="ones3")
            nc.vector.memset(ones3[:], 1.0)
            for g in range(2):
                nc.vector.tensor_copy(v1[g][:, :, 0:HD], ones3[:])

            # ---------------- stage-A unit generators ----------------
            pend = []           # deferred PE units (rot matmuls, V transposes)

            def rope_chain(dst, src_ps, ssl):
                """RoPE: the rotate-half partition swap runs ON THE PE as
                one matmul with a constant permutation matrix (213 ns) --
                SBUF->SBUF swap DMAs get starved for 20-30 us behind the
                HBM weight/x streams.  The rot matmul is deferred (pend)
                so the PE never head-of-line-waits on the psum drain.
                The sign of the rotation is folded into sinT host-side."""
                raw = pc.tile([P, QT], BF16, tag="raw", bufs=5, name="raw")
                nc.vector.tensor_copy(raw[:], src_ps)

                def rot_unit():
                    rps = aqb("rope_ps")
                    nc.tensor.matmul(rps[:], rope_mat[:], raw[:],
                                     start=True, stop=True)
                    rot = pc.tile([P, QT], BF16, tag="rot", bufs=4,
                                  name="rot")
                    # drain on ACT: a DVE drain here would head-of-line
                    # block the DVE FIFO on the (deferred) rot matmul
                    nc.scalar.copy(rot[:], rps[:])
                    nc.vector.tensor_mul(rot[:], rot[:], sin_sb[:, ssl])
                    nc.vector.tensor_mul(raw[:], raw[:], cos_sb[:, ssl])
                    nc.vector.tensor_add(dst[:, ssl], raw[:], rot[:])
                pend.append(rot_unit)

            # pass order within a s-tile: k first, then v, then q pairs --
            # the first attention group of the next phase consumes k_fin,
            # v1 and q_fin[0], in that order, so their ropes/transposes
            # must complete earliest (else the first score matmul
            # head-of-line-blocks the PE queue at the phase boundary).
            PASS_ORDER = (4, 5, 0, 1, 2, 3)

            def a_pass(st, which):
                """One projection pass for s-tile st: 16 accumulating
                matmuls into a single psum bank, then drain.  which:
                0-3 = q pair, 4 = k, 5 = v."""
                ssl = slice(st * QT, (st + 1) * QT)
                acc = aqb(f"ap_{st}_{which}")
                w = (wq_sb[which] if which < 4 else
                     (wk_sb if which == 4 else wv_sb))
                for o in range(DK):
                    nc.tensor.matmul(acc[:], w[:, o, :],
                                     xsl[st, o // OCH][:, o % OCH, :],
                                     start=(o == 0), stop=(o == DK - 1))
                if which < 4:
                    rope_chain(q_fin[which], acc[:], ssl)
                elif which == 4:
                    rope_chain(k_fin, acc[:], ssl)
                else:
                    # drain per 128-col chunk; defer the PE transposes so
                    # they never stall the PE queue on the ACT drain.
                    for cj, j in enumerate(range(4 * st, 4 * st + 4)):
                        nc.scalar.copy(vT_raw[:, j * P:(j + 1) * P],
                                       acc[:, cj * P:(cj + 1) * P])
                        pend.append(lambda j=j: v_transpose(j))

            def v_transpose(j):
                pst = aqb(f"pst{j}", [P, P], BF16)
                nc.tensor.transpose(
                    pst[:], vT_raw[:, j * P:(j + 1) * P], ident[:])
                for g in range(2):
                    nc.vector.tensor_copy(
                        v1[g][:, j, HD:P], pst[:, g * HD:(g + 1) * HD])

            def flush_tr(nmax=1):
                for _ in range(min(nmax, len(pend))):
                    pend.pop(0)()

            # ---------------- attention unit generators ----------------
            prs = [slice(0, HD), slice(HD, P)]
            pair_state = {}

            def scores_exp(t, m, g2, qo, qw):
                """Score pair + exp + (mask) for kv group g2 of head
                pair m at q tile t, over q columns [qo, qo+qw) of the
                tile.  Returns the two e tiles (i = kv tile in group)."""
                st8 = pair_state[t, m]
                qsl = slice(t * QT + qo, t * QT + qo + qw)
                diag = (g2 == 2 * t)            # needs masking
                half = (g2 == 2 * t + 1)        # diagonal half group
                es = []
                for i in range(2):
                    j = 2 * g2 + i
                    pss = sc2(f"ss_{t}_{m}_{g2}_{i}")
                    for hf in range(2):
                        nc.tensor.matmul(
                            pss[:, hf, 0:qw],
                            k_fin[prs[hf], j * P:(j + 1) * P],
                            q_fin[m][prs[hf], qsl],
                            start=True, stop=True)
                    e = pc.tile([P, 2, qw], BF16,
                                tag="exp" if qw == QT else "exph",
                                bufs=10 if qw == QT else 6, name="e2")
                    nc.scalar.activation(e[:], pss[:, :, 0:qw], Exp,
                                         scale=0.125)
                    if diag:
                        nc.vector.tensor_mul(
                            e[:, :, 0:QH], e[:, :, 0:QH], msk[:, i])
                    elif half:
                        nc.vector.tensor_mul(e[:], e[:], msk[:, i])
                    es.append(e)
                st8["e"].append((es, qo, qw))

            def attn_group(t, m, g2):
                scores_exp(t, m, g2, 0, QT)
                if g2 >= 1:
                    _pv_flush(t, m, g2 - 1)

            def attn_half(t, m):
                scores_exp(t, m, 2 * t + 1, QH, QH)
                _pv_flush(t, m, 2 * t)

            def _pv_flush(t, m, gp):
                st8 = pair_state[t, m]
                last_j = 4 * t + 3
                es, qo, qw = st8["e"][gp]
                for i in range(2):
                    j = 2 * gp + i
                    for hf in range(2):
                        nc.tensor.matmul(
                            st8["pv"][hf][:, qo:qo + qw], v1[hf][:, j, :],
                            es[i][:, hf, :],
                            start=(j == 0), stop=(j == last_j))

            def attn_norm(t, m):
                st8 = pair_state[t, m]
                _pv_flush(t, m, 2 * t + 1)
                qsl = slice(t * QT, (t + 1) * QT)
                for hf in range(2):
                    recip = pc.tile([HD, QT], F32, tag="recip", bufs=2,
                                    name="recip")
                    nc.vector.reciprocal_approx_fast(
                        recip[:], st8["pv"][hf][0:HD, :])
                    nc.vector.tensor_mul(
                        a_fin[m][hf * HD:(hf + 1) * HD, qsl],
                        st8["pv"][hf][HD:P, :], recip[:])

            def wo_pair(t, dp, tail=False):
                """Partial wo for q tile t, output d-pair dp (2 x 128
                dims): contract over the local 512 attn features.  Two
                1-bank psum chunks drain into one SBUF tile -> one DMA
                (two in the tail, so the store stream starts earlier)."""
                qsl = slice(t * QT, (t + 1) * QT)
                ot = pc.tile([P, 2, QT], BF16, tag="ot", bufs=4, name="ot")
                # tail chunks allocate from the sc2 banks (free once the
                # scores are done) -> 4-deep rotation instead of fighting
                # the aq tag for 2 banks
                pwt = sc2(f"wot_{t}_{dp}") if tail else None
                for dd in range(2):
                    d = 2 * dp + dd
                    pw = pwt[:, dd, :] if tail else aqb(f"wo_{t}_{d}")[:]
                    for f in range(4):
                        nc.tensor.matmul(
                            pw, wo_sb[:, f, d * P:(d + 1) * P],
                            a_fin[f][:, qsl], start=(f == 0), stop=(f == 3))
                    if tail and dd == 1:
                        nc.scalar.copy(ot[:, dd, :], pw)
                    else:
                        nc.vector.tensor_copy(ot[:, dd, :], pw)
                    if tail:
                        dq = nc.scalar if dd == 1 else nc.sync
                        dq.dma_start(out_td[:, t, d:d + 1, :],
                                     ot[:, dd:dd + 1, :])
                if not tail:
                    nc.sync.dma_start(out_td[:, t, 2 * dp:2 * dp + 2, :],
                                      ot[:])

            # ---------------- woven schedule ----------------
            # prologue: s-tile 0 projections
            for w in PASS_ORDER:
                a_pass(0, w)
                flush_tr(2)

            for t in range(nqt):
                # c-units: attention groups + per-pair normalize
                c_units = []
                for m in range(4):
                    pair_state[t, m] = {
                        "pv": [pvb(f"pv_{t}_{m}_{hf}") for hf in range(2)],
                        "e": []}
                    for g2 in range(2 * t + 1):
                        c_units.append(
                            lambda t=t, m=m, g2=g2: attn_group(t, m, g2))
                    c_units.append(lambda t=t, m=m: attn_half(t, m))
                    c_units.append(lambda t=t, m=m: attn_norm(t, m))
                # filler units: wo chunks of tile t-1, projection passes
                # of s-tile t+1 (x chunks DMA-kicked first)
                f_units = []
                if t + 1 < nqt:
                    f_units.append(lambda st=t + 1: x_load(st))
                    for w in PASS_ORDER:
                        f_units.append(lambda st=t + 1, w=w: a_pass(st, w))
                if t >= 1:
                    for dp in range(DK // 2):
                        f_units.append(lambda t=t - 1, dp=dp: wo_pair(t, dp))
                # interleave: spread fillers evenly between c-units
                nf, ncu = len(f_units), len(c_units)
                fi = 0
                for ci, cu in enumerate(c_units):
                    cu()
                    flush_tr(1)
                    while fi < nf and fi * ncu < (ci + 1) * nf:
                        f_units[fi]()
                        flush_tr(1)
                        fi += 1
                while fi < nf:
                    f_units[fi]()
                    fi += 1
                flush_tr(99)
            # tail: last tile's wo; second drain of each pair on the
            # (now idle) scalar engine so drains overlap the matmuls
            for dp in range(DK // 2):
                wo_pair(nqt - 1, dp, tail=True)

    nc.compile()
    return nc


def _prep_inputs(x, position_ids, wq, wk, wv, wo):
    import ml_dtypes

    bf16 = ml_dtypes.bfloat16
    x = np.asarray(x, dtype=np.float32)
    pos = np.asarray(position_ids).reshape(-1).astype(np.int64)
    wqf = np.asarray(wq, dtype=np.float32)
    wkf = np.asarray(wk, dtype=np.float32)
    wvf = np.asarray(wv, dtype=np.float32)
    wof = np.asarray(wo, dtype=np.float32)

    inv = 1.0 / (ROPE_BASE ** (np.arange(0, HD, 2, dtype=np.float32) / HD))
    freqs = np.outer(pos.astype(np.float32), inv)  # [S, 32]
    pidx = np.arange(P) % 32
    sign = np.where((np.arange(P) % HD) < 32, -1.0, 1.0).astype(np.float32)
    cosT = np.ascontiguousarray(np.cos(freqs)[:, pidx].T).astype(bf16)
    sinT = np.ascontiguousarray(
        np.sin(freqs)[:, pidx].T * sign[:, None]).astype(bf16)

    # mask[p, i, hf, f] = (f - p - 128*i >= 0), duplicated over hf
    pg = np.arange(P)[:, None, None, None]
    ig = np.arange(2)[None, :, None, None]
    fg = np.arange(QH)[None, None, None, :]
    maskT = np.broadcast_to((fg - pg - 128 * ig) >= 0,
                            (P, 2, 2, QH)).astype(bf16)

    # rotate-half permutation matrix: out[m] = in[m ^ 32]
    rotmT = np.zeros((P, P), dtype=bf16)
    ii = np.arange(P)
    rotmT[ii, ii ^ 32] = 1.0

    # device layouts: everything pre-tiled so each DMA is a contiguous
    # 2-4 KiB run per partition (see _build).
    DK, NXC, OCH, nst = DIM // P, 4, 4, S // QT

    def tile_w(wT_loc, ncol_tiles):
        # [DIM, ncol_tiles*128] -> [p, m, o, col] (m = 128-col tile)
        a = wT_loc.reshape(DK, P, ncol_tiles, P)
        return np.ascontiguousarray(a.transpose(1, 2, 0, 3))

    xT4 = []
    for b in range(B):
        xb = np.ascontiguousarray(x[b].T).astype(bf16)   # [D, S]
        a = xb.reshape(NXC, OCH, P, nst, QT)
        xT4.append(np.ascontiguousarray(
            a.transpose(2, 3, 0, 1, 4)).reshape(P, nst, NXC, OCH * QT))

    in_maps = []
    for c in range(N_CORES):
        b, k = c // 4, c % 4
        # q columns: pair i holds head 8k+i (cols 0:64 of the pair) and
        # head 8k+4+i (cols 64:128)
        qcols = np.concatenate(
            [np.arange(64 * (8 * k + i + 4 * hf), 64 * (8 * k + i + 4 * hf) + 64)
             for i in range(4) for hf in range(2)])
        wqT_loc = wqf[qcols].T.astype(bf16)
        kvcols = np.arange(64 * 2 * k, 64 * (2 * k + 2))
        wkT_loc = wkf[kvcols].T.astype(bf16)
        wvT_loc = wvf[kvcols].T.astype(bf16)
        # wo rows in the a_fin feature order (f = 128*i + 64*hf + d)
        woT_loc = wof[:, qcols].T.astype(bf16)
        in_maps.append({
            "xT4": xT4[b],
            "wqT4": tile_w(wqT_loc, 4),
            "wkT3": tile_w(wkT_loc, 1).reshape(P, DK, P),
            "wvT3": tile_w(wvT_loc, 1).reshape(P, DK, P),
            "woT3": np.ascontiguousarray(
                woT_loc.reshape(4, P, DIM).transpose(1, 0, 2)),
            "cosT": cosT,
            "sinT": sinT,
            "maskT": np.ascontiguousarray(maskT),
            "rotmT": rotmT,
        })
    return in_maps


LAST_EXEC_NS = None


def kernel(x, position_ids, wq, wk, wv, wo, _trace=False):
    import time

    from concourse import bass_utils

    if "nc" not in _CACHE:
        _CACHE["nc"] = _build()
    nc = _CACHE["nc"]

    in_maps = _prep_inputs(x, position_ids, wq, wk, wv, wo)
    res = None
    for attempt in range(3):
        try:
            res = bass_utils.run_bass_kernel_spmd(
                nc, in_maps, core_ids=list(range(N_CORES)), trace=_trace)
            break
        except Exception:
            # transient device hiccups (e.g. NRT_EXEC_UNIT_UNRECOVERABLE
            # after rapid back-to-back runs) usually clear on retry
            if attempt == 2:
                raise
            time.sleep(20 * (attempt + 1))

    global LAST_EXEC_NS
    LAST_EXEC_NS = res.exec_time_ns

    out = np.zeros((B, S, DIM), dtype=np.float32)
    for c in range(N_CORES):
        b = c // 4
        arr = res.results[c]["out_td"].astype(np.float32)   # [P,nst,DK,QT]
        out[b] += arr.transpose(2, 0, 1, 3).reshape(DIM, S).T
    return out


# revision 31
# speedup vs baseline: 1.0195x; 1.0195x over previous
"""Trainium2 Bass kernel for causal GQA attention (B=2, S=2048, D=2048,
H=32, KVH=8, hd=64) with RoPE and output projection, running SPMD on 8
NeuronCores.

Sharding: tensor-parallel over heads (4-way) x data-parallel over batch
(2-way).  Core c (b = c//4, k = c%4) handles batch b and heads
8k..8k+8 (kv heads 2k, 2k+1).  No collectives: each core computes a
PARTIAL wo product (contraction over its local 512 attention features,
all 2048 output dims) and the host sums the 4 partials per batch.

Layouts: everything lives in transposed [feature, seq] form so that the
head dim (the contraction dim of QK^T) sits on SBUF partitions and no
on-device transposes are required (except a cheap PE transpose for V).
All matmul operands are bf16 (stationary bf16 enables fast weight load
so LDWEIGHTS hides under the matmuls); accumulation is fp32 in PSUM.

Head pairing: q_fin[i] holds head 8k+i (kv head 2k) on partitions 0:64
and head 8k+4+i (kv head 2k+1) on partitions 64:128, so the two packed
score matmuls of a pair use the two DIFFERENT kv heads and the K
projection needs no duplication.

Score-matmul concurrency: the two half-head (contraction-64) score
matmuls of a kv tile sit on disjoint PE row groups (base partitions 0
and 64), so the hardware can run them CONCURRENTLY -- but only if they
are issued back-to-back.  To make that structural, each kv tile i gets
ONE psum tile [P, 2(hf), QT]; both hf matmuls write different banks of
it and become ready on the same exp-drain event, so they always emit
adjacently and overlap (~2x on the score phase).

Causal diagonal trimming: for q tile t, the last kv group (columns
t*512+256 .. (t+1)*512) is only needed by the second half of the q
rows, so its scores/exp/PV run at free dim 256.  The mask multiply
only ever touches the first 256 q columns of a group.

Schedule: the projection work for s-tile t+1 and the wo matmuls for
q-tile t-1 are WOVEN between the attention score groups of q-tile t, so
the PE has dense matmul work while the scalar engine (exp, the
second-busiest engine) chews through the softmax.  V transposes are
deferred a unit so they never head-of-line-block the PE queue.  PSUM:
score tag 2x2 banks + PV 2x1 + a shared 1-bank tag for projection
passes / wo chunks x2 = 8 banks.
"""

import numpy as np

DIM = 2048
S = 2048
B = 2
H = 32
KVH = 8
HD = 64
P = 128
QT = 512        # q tile (free dim of score matmuls)
QH = QT // 2    # half tile for the causal-diagonal group
ROPE_BASE = 10000.0
N_CORES = 8

_CACHE = {}


def _build(s_len=S):
    import concourse.bacc as bacc
    import concourse.tile as tile
    import concourse.mybir as mybir
    from concourse.masks import make_identity

    F32 = mybir.dt.float32
    BF16 = mybir.dt.bfloat16
    Exp = mybir.ActivationFunctionType.Exp

    nqt = s_len // QT      # q tiles
    nkv = s_len // P       # kv tiles of 128
    DK = DIM // P          # 16 contraction tiles for projections
    NXC = 4                # x chunks per s-tile
    OCH = DK // NXC        # 4 contraction 128-tiles per x chunk

    nc = bacc.Bacc("TRN2", target_bir_lowering=False, debug=False,
                   num_devices=N_CORES)

    # All DRAM tensors are HOST-PRE-TILED so that every DMA reads/writes
    # a long contiguous run per partition (2-4 KiB packets); the naive
    # [feature, seq] layouts generate 256 B packets and run the HWDGE
    # queues at <100 GB/s, which starves the prologue.
    nst = s_len // QT
    xT4 = nc.dram_tensor("xT4", [P, nst, NXC, OCH * QT], BF16,
                         kind="ExternalInput").ap()
    wqT4 = nc.dram_tensor("wqT4", [P, 4, DK, P], BF16,
                          kind="ExternalInput").ap()
    wkT3 = nc.dram_tensor("wkT3", [P, DK, P], BF16,
                          kind="ExternalInput").ap()
    wvT3 = nc.dram_tensor("wvT3", [P, DK, P], BF16,
                          kind="ExternalInput").ap()
    woT3 = nc.dram_tensor("woT3", [P, 4, DIM], BF16,
                          kind="ExternalInput").ap()
    cosT = nc.dram_tensor("cosT", [P, s_len], BF16, kind="ExternalInput").ap()
    sinT = nc.dram_tensor("sinT", [P, s_len], BF16, kind="ExternalInput").ap()
    maskT = nc.dram_tensor("maskT", [P, 2, 2, QH], BF16,
                           kind="ExternalInput").ap()
    rotmT = nc.dram_tensor("rotmT", [P, P], BF16, kind="ExternalInput").ap()
    out_td = nc.dram_tensor("out_td", [P, nst, DK, QT], BF16,
                            kind="ExternalOutput").ap()

    with tile.TileContext(nc) as tc:
        with (
            tc.tile_pool(name="pers", bufs=1) as pers,
            tc.tile_pool(name="ps", bufs=1, space="PSUM") as ps,
            tc.tile_pool(name="pc", bufs=1) as pc,
        ):
            # ---- persistent tiles ----
            q_fin = [pers.tile([P, s_len], BF16, name=f"q_fin{m}")
                     for m in range(4)]
            k_fin = pers.tile([P, s_len], BF16, name="k_fin")
            v1 = [pers.tile([P, nkv, P], BF16, name=f"v1_{g}")
                  for g in range(2)]
            a_fin = [pers.tile([P, s_len], BF16, name=f"a_fin{i}")
                     for i in range(4)]
            msk = pers.tile([P, 2, 2, QH], BF16, name="msk")
            vT_raw = pers.tile([P, s_len], BF16, name="vT_raw")
            wq_sb = [pers.tile([P, DK, P], BF16, name=f"wq_sb{m}")
                     for m in range(4)]
            wk_sb = pers.tile([P, DK, P], BF16, name="wk_sb")
            wv_sb = pers.tile([P, DK, P], BF16, name="wv_sb")
            wo_sb = pers.tile([P, 4, DIM], BF16, name="wo_sb")
            cos_sb = pers.tile([P, s_len], BF16, name="cos_sb")
            sin_sb = pers.tile([P, s_len], BF16, name="sin_sb")
            ident = pers.tile([P, P], BF16, name="ident")
            rope_mat = pers.tile([P, P], BF16, name="rope_mat")

            # PSUM tags (8 banks): sc2 = 2 tiles x 2 banks (score kv
            # tiles: [hf, q]), pv = 2 tiles x 1 bank (PV accum), aq = 2
            # tiles x 1 bank (projection passes, wo chunks, V
            # transposes).
            def sc2(name):
                return ps.tile([P, 2, QT], F32, tag="sc2", bufs=2, name=name)

            def pvb(name):
                return ps.tile([P, QT], F32, tag="pv", bufs=2, name=name)

            def aqb(name, shape=None, dtype=None):
                return ps.tile(shape or [P, QT], dtype or F32, tag="aq",
                               bufs=2, name=name)

            # ---------------- prologue DMAs ----------------
            # x chunks stream on the sync HWDGE queue; everything else on
            # the scalar HWDGE queue so the x stream is never stuck behind
            # 6 MB of weights.  Weight order matches the prologue pass
            # order (k, v, q0..q3): wk first so pass k starts ~1.5us in,
            # cos/sin right behind it so the k rope (which gates the first
            # score matmul) is never the long pole.
            xsl = {}

            def x_load(st):
                for cn in range(NXC):
                    t_ = pc.tile([P, OCH, QT], BF16, tag="xsl",
                                 bufs=2 * NXC, name=f"x{st}_{cn}")
                    nc.sync.dma_start(
                        t_[:],
                        xT4[:, st, cn].rearrange("p (o q) -> p o q", o=OCH))
                    xsl[st, cn] = t_

            x_load(0)
            nc.scalar.dma_start(wk_sb[:], wkT3[:])
            nc.scalar.dma_start(rope_mat[:], rotmT[:])
            nc.scalar.dma_start(cos_sb[:], cosT[:])
            nc.scalar.dma_start(sin_sb[:], sinT[:])
            nc.scalar.dma_start(wv_sb[:], wvT3[:])
            nc.scalar.dma_start(wq_sb[0][:], wqT4[:, 0])
            nc.scalar.dma_start(msk[:], maskT[:])
            for m in range(1, 4):
                nc.scalar.dma_start(wq_sb[m][:], wqT4[:, m])
            nc.scalar.dma_start(wo_sb[:], woT3[:])
            ident_f = pc.tile([P, P], F32, name="ident_f")
            make_identity(nc, ident_f[:])
            nc.vector.tensor_copy(ident[:], ident_f[:])
            ones3 = pc.tile([P, nkv, HD], F32, name="ones3")
            nc.vector.memset(ones3[:], 1.0)
            for g in range(2):
                nc.vector.tensor_copy(v1[g][:, :, 0:HD], ones3[:])

            # ---------------- stage-A unit generators ----------------
            pend = []           # deferred PE units (rot matmuls, V transposes)

            def rope_chain(dst, src_ps, ssl):
                """RoPE: the rotate-half partition swap runs ON THE PE as
                one matmul with a constant permutation matrix (213 ns) --
                SBUF->SBUF swap DMAs get starved for 20-30 us behind the
                HBM weight/x streams.  The rot matmul is deferred (pend)
                so the PE never head-of-line-waits on the psum drain.
                The sign of the rotation is folded into sinT host-side."""
                raw = pc.tile([P, QT], BF16, tag="raw", bufs=5, name="raw")
                nc.vector.tensor_copy(raw[:], src_ps)

                def rot_unit():
                    rps = aqb("rope_ps")
                    nc.tensor.matmul(rps[:], rope_mat[:], raw[:],
                                     start=True, stop=True)
                    rot = pc.tile([P, QT], BF16, tag="rot", bufs=4,
                                  name="rot")
                    # drain on ACT: a DVE drain here would head-of-line
                    # block the DVE FIFO on the (deferred) rot matmul
                    nc.scalar.copy(rot[:], rps[:])
                    nc.vector.tensor_mul(rot[:], rot[:], sin_sb[:, ssl])
                    nc.vector.tensor_mul(raw[:], raw[:], cos_sb[:, ssl])
                    nc.vector.tensor_add(dst[:, ssl], raw[:], rot[:])
                pend.append(rot_unit)

            # pass order within a s-tile: k first, then v, then q pairs --
            # the first attention group of the next phase consumes k_fin,
            # v1 and q_fin[0], in that order, so their ropes/transposes
            # must complete earliest (else the first score matmul
            # head-of-line-blocks the PE queue at the phase boundary).
            PASS_ORDER = (4, 5, 0, 1, 2, 3)

            def a_pass(st, which):
                """One projection pass for s-tile st: 16 accumulating
                matmuls into a single psum bank, then drain.  which:
                0-3 = q pair, 4 = k, 5 = v."""
                ssl = slice(st * QT, (st + 1) * QT)
                acc = aqb(f"ap_{st}_{which}")
                w = (wq_sb[which] if which < 4 else
                     (wk_sb if which == 4 else wv_sb))
                for o in range(DK):
                    nc.tensor.matmul(acc[:], w[:, o, :],
                                     xsl[st, o // OCH][:, o % OCH, :],
                                     start=(o == 0), stop=(o == DK - 1))
                if which < 4:
                    rope_chain(q_fin[which], acc[:], ssl)
                elif which == 4:
                    rope_chain(k_fin, acc[:], ssl)
                else:
                    # drain per 128-col chunk; defer the PE transposes so
                    # they never stall the PE queue on the ACT drain.
                    for cj, j in enumerate(range(4 * st, 4 * st + 4)):
                        nc.scalar.copy(vT_raw[:, j * P:(j + 1) * P],
                                       acc[:, cj * P:(cj + 1) * P])
                        pend.append(lambda j=j: v_transpose(j))

            def v_transpose(j):
                pst = aqb(f"pst{j}", [P, P], BF16)
                nc.tensor.transpose(
                    pst[:], vT_raw[:, j * P:(j + 1) * P], ident[:])
                for g in range(2):
                    nc.vector.tensor_copy(
                        v1[g][:, j, HD:P], pst[:, g * HD:(g + 1) * HD])

            def flush_tr(nmax=1):
                for _ in range(min(nmax, len(pend))):
                    pend.pop(0)()

            # ---------------- attention unit generators ----------------
            prs = [slice(0, HD), slice(HD, P)]
            pair_state = {}

            def scores_exp(t, m, g2, qo, qw):
                """Score pair + exp + (mask) for kv group g2 of head
                pair m at q tile t, over q columns [qo, qo+qw) of the
                tile.  Returns the two e tiles (i = kv tile in group)."""
                st8 = pair_state[t, m]
                qsl = slice(t * QT + qo, t * QT + qo + qw)
                diag = (g2 == 2 * t)            # needs masking
                half = (g2 == 2 * t + 1)        # diagonal half group
                es = []
                for i in range(2):
                    j = 2 * g2 + i
                    pss = sc2(f"ss_{t}_{m}_{g2}_{i}")
                    for hf in range(2):
                        nc.tensor.matmul(
                            pss[:, hf, 0:qw],
                            k_fin[prs[hf], j * P:(j + 1) * P],
                            q_fin[m][prs[hf], qsl],
                            start=True, stop=True)
                    e = pc.tile([P, 2, qw], BF16,
                                tag="exp" if qw == QT else "exph",
                                bufs=10 if qw == QT else 6, name="e2")
                    nc.scalar.activation(e[:], pss[:, :, 0:qw], Exp,
                                         scale=0.125)
                    if diag:
                        nc.vector.tensor_mul(
                            e[:, :, 0:QH], e[:, :, 0:QH], msk[:, i])
                    elif half:
                        nc.vector.tensor_mul(e[:], e[:], msk[:, i])
                    es.append(e)
                st8["e"].append((es, qo, qw))

            def attn_group(t, m, g2):
                scores_exp(t, m, g2, 0, QT)
                if g2 >= 1:
                    _pv_flush(t, m, g2 - 1)

            def attn_half(t, m):
                scores_exp(t, m, 2 * t + 1, QH, QH)
                _pv_flush(t, m, 2 * t)

            def _pv_flush(t, m, gp):
                st8 = pair_state[t, m]
                last_j = 4 * t + 3
                es, qo, qw = st8["e"][gp]
                for i in range(2):
                    j = 2 * gp + i
                    for hf in range(2):
                        nc.tensor.matmul(
                            st8["pv"][hf][:, qo:qo + qw], v1[hf][:, j, :],
                            es[i][:, hf, :],
                            start=(j == 0), stop=(j == last_j))

            def attn_norm(t, m):
                st8 = pair_state[t, m]
                _pv_flush(t, m, 2 * t + 1)
                qsl = slice(t * QT, (t + 1) * QT)
                for hf in range(2):
                    recip = pc.tile([HD, QT], F32, tag="recip", bufs=2,
                                    name="recip")
                    nc.vector.reciprocal_approx_fast(
                        recip[:], st8["pv"][hf][0:HD, :])
                    nc.vector.tensor_mul(
                        a_fin[m][hf * HD:(hf + 1) * HD, qsl],
                        st8["pv"][hf][HD:P, :], recip[:])

            def wo_pair(t, dp, tail=False):
                """Partial wo for q tile t, output d-pair dp (2 x 128
                dims): contract over the local 512 attn features.  Two
                1-bank psum chunks drain into one SBUF tile -> one DMA
                (two in the tail, so the store stream starts earlier)."""
                qsl = slice(t * QT, (t + 1) * QT)
                ot = pc.tile([P, 2, QT], BF16, tag="ot", bufs=4, name="ot")
                # tail chunks allocate from the sc2 banks (free once the
                # scores are done) -> 4-deep rotation instead of fighting
                # the aq tag for 2 banks
                pwt = sc2(f"wot_{t}_{dp}") if tail else None
                for dd in range(2):
                    d = 2 * dp + dd
                    pw = pwt[:, dd, :] if tail else aqb(f"wo_{t}_{d}")[:]
                    for f in range(4):
                        nc.tensor.matmul(
                            pw, wo_sb[:, f, d * P:(d + 1) * P],
                            a_fin[f][:, qsl], start=(f == 0), stop=(f == 3))
                    if tail and dd == 1:
                        nc.scalar.copy(ot[:, dd, :], pw)
                    else:
                        nc.vector.tensor_copy(ot[:, dd, :], pw)
                    if tail:
                        nc.sync.dma_start(out_td[:, t, d:d + 1, :],
                                          ot[:, dd:dd + 1, :])
                if not tail:
                    nc.sync.dma_start(out_td[:, t, 2 * dp:2 * dp + 2, :],
                                      ot[:])

            # ---------------- woven schedule ----------------
            # prologue: s-tile 0 projections
            for w in PASS_ORDER:
                a_pass(0, w)
                flush_tr(2)

            for t in range(nqt):
                # c-units: attention groups + per-pair normalize
                c_units = []
                for m in range(4):
                    pair_state[t, m] = {
                        "pv": [pvb(f"pv_{t}_{m}_{hf}") for hf in range(2)],
                        "e": []}
                    for g2 in range(2 * t + 1):
                        c_units.append(
                            lambda t=t, m=m, g2=g2: attn_group(t, m, g2))
                    c_units.append(lambda t=t, m=m: attn_half(t, m))
                    c_units.append(lambda t=t, m=m: attn_norm(t, m))
                # filler units: wo chunks of tile t-1, projection passes
                # of s-tile t+1 (x chunks DMA-kicked first)
                f_units = []
                if t + 1 < nqt:
                    f_units.append(lambda st=t + 1: x_load(st))
                    for w in PASS_ORDER:
                        f_units.append(lambda st=t + 1, w=w: a_pass(st, w))
                if t >= 1:
                    for dp in range(DK // 2):
                        f_units.append(lambda t=t - 1, dp=dp: wo_pair(t, dp))
                # interleave: spread fillers evenly between c-units
                nf, ncu = len(f_units), len(c_units)
                fi = 0
                for ci, cu in enumerate(c_units):
                    cu()
                    flush_tr(1)
                    while fi < nf and fi * ncu < (ci + 1) * nf:
                        f_units[fi]()
                        flush_tr(1)
                        fi += 1
                while fi < nf:
                    f_units[fi]()
                    fi += 1
                flush_tr(99)
            # tail: last tile's wo; second drain of each pair on the
            # (now idle) scalar engine so drains overlap the matmuls
            for dp in range(DK // 2):
                wo_pair(nqt - 1, dp, tail=True)

    nc.compile()
    return nc


def _prep_inputs(x, position_ids, wq, wk, wv, wo):
    import ml_dtypes

    bf16 = ml_dtypes.bfloat16
    x = np.asarray(x, dtype=np.float32)
    pos = np.asarray(position_ids).reshape(-1).astype(np.int64)
    wqf = np.asarray(wq, dtype=np.float32)
    wkf = np.asarray(wk, dtype=np.float32)
    wvf = np.asarray(wv, dtype=np.float32)
    wof = np.asarray(wo, dtype=np.float32)

    inv = 1.0 / (ROPE_BASE ** (np.arange(0, HD, 2, dtype=np.float32) / HD))
    freqs = np.outer(pos.astype(np.float32), inv)  # [S, 32]
    pidx = np.arange(P) % 32
    sign = np.where((np.arange(P) % HD) < 32, -1.0, 1.0).astype(np.float32)
    cosT = np.ascontiguousarray(np.cos(freqs)[:, pidx].T).astype(bf16)
    sinT = np.ascontiguousarray(
        np.sin(freqs)[:, pidx].T * sign[:, None]).astype(bf16)

    # mask[p, i, hf, f] = (f - p - 128*i >= 0), duplicated over hf
    pg = np.arange(P)[:, None, None, None]
    ig = np.arange(2)[None, :, None, None]
    fg = np.arange(QH)[None, None, None, :]
    maskT = np.broadcast_to((fg - pg - 128 * ig) >= 0,
                            (P, 2, 2, QH)).astype(bf16)

    # rotate-half permutation matrix: out[m] = in[m ^ 32]
    rotmT = np.zeros((P, P), dtype=bf16)
    ii = np.arange(P)
    rotmT[ii, ii ^ 32] = 1.0

    # device layouts: everything pre-tiled so each DMA is a contiguous
    # 2-4 KiB run per partition (see _build).
    DK, NXC, OCH, nst = DIM // P, 4, 4, S // QT

    def tile_w(wT_loc, ncol_tiles):
        # [DIM, ncol_tiles*128] -> [p, m, o, col] (m = 128-col tile)
        a = wT_loc.reshape(DK, P, ncol_tiles, P)
        return np.ascontiguousarray(a.transpose(1, 2, 0, 3))

    xT4 = []
    for b in range(B):
        xb = np.ascontiguousarray(x[b].T).astype(bf16)   # [D, S]
        a = xb.reshape(NXC, OCH, P, nst, QT)
        xT4.append(np.ascontiguousarray(
            a.transpose(2, 3, 0, 1, 4)).reshape(P, nst, NXC, OCH * QT))

    in_maps = []
    for c in range(N_CORES):
        b, k = c // 4, c % 4
        # q columns: pair i holds head 8k+i (cols 0:64 of the pair) and
        # head 8k+4+i (cols 64:128)
        qcols = np.concatenate(
            [np.arange(64 * (8 * k + i + 4 * hf), 64 * (8 * k + i + 4 * hf) + 64)
             for i in range(4) for hf in range(2)])
        wqT_loc = wqf[qcols].T.astype(bf16)
        kvcols = np.arange(64 * 2 * k, 64 * (2 * k + 2))
        wkT_loc = wkf[kvcols].T.astype(bf16)
        wvT_loc = wvf[kvcols].T.astype(bf16)
        # wo rows in the a_fin feature order (f = 128*i + 64*hf + d)
        woT_loc = wof[:, qcols].T.astype(bf16)
        in_maps.append({
            "xT4": xT4[b],
            "wqT4": tile_w(wqT_loc, 4),
            "wkT3": tile_w(wkT_loc, 1).reshape(P, DK, P),
            "wvT3": tile_w(wvT_loc, 1).reshape(P, DK, P),
            "woT3": np.ascontiguousarray(
                woT_loc.reshape(4, P, DIM).transpose(1, 0, 2)),
            "cosT": cosT,
            "sinT": sinT,
            "maskT": np.ascontiguousarray(maskT),
            "rotmT": rotmT,
        })
    return in_maps


LAST_EXEC_NS = None


def kernel(x, position_ids, wq, wk, wv, wo, _trace=False):
    import time

    from concourse import bass_utils

    if "nc" not in _CACHE:
        _CACHE["nc"] = _build()
    nc = _CACHE["nc"]

    in_maps = _prep_inputs(x, position_ids, wq, wk, wv, wo)
    res = None
    for attempt in range(3):
        try:
            res = bass_utils.run_bass_kernel_spmd(
                nc, in_maps, core_ids=list(range(N_CORES)), trace=_trace)
            break
        except Exception:
            # transient device hiccups (e.g. NRT_EXEC_UNIT_UNRECOVERABLE
            # after rapid back-to-back runs) usually clear on retry
            if attempt == 2:
                raise
            time.sleep(20 * (attempt + 1))

    global LAST_EXEC_NS
    LAST_EXEC_NS = res.exec_time_ns

    out = np.zeros((B, S, DIM), dtype=np.float32)
    for c in range(N_CORES):
        b = c // 4
        arr = res.results[c]["out_td"].astype(np.float32)   # [P,nst,DK,QT]
        out[b] += arr.transpose(2, 0, 1, 3).reshape(DIM, S).T
    return out
